# revision 1
# baseline (speedup 1.0000x reference)
"""Trainium2 Bass kernel for nn_Attention_37074157699274.

Multi-head self-attention over tiny 32-token groups:
  x [4, 1024, 32, 256] -> qkv -> per-(b,p)-group 8-head attention -> out proj.

Strategy: data-parallel over the 4096 (b,p) groups across 8 NeuronCores
(512 groups/core).  On-core, groups are processed in blocks of 4 (=128
tokens, one partition span).  Per block:
  - DMA x block [128,256] f32, cast bf16, DMA-xbar-transpose to xT.
  - QKV projection on PE: q,k produced feature-major ([feat,tok], so heads
    land at partition offsets usable as matmul tiles), v token-major.
  - dots via 32 tiny matmuls packed with PE tile_position (K=64,M=32,N=32).
  - softmax on ACT (exp, fused *0.125 scale) + DVE (segmented sum, recip,
    normalize) -- compact [128, 8*32], no masking waste.
  - attn 32x32 block-transpose on DVE stream-transpose.
  - attn@v as 32 tiny matmuls producing oT (inner-major) directly.
  - out projection consumes oT chunks as stationary operands; bias added
    during PSUM evacuation on DVE; DMA out.
"""

import numpy as np

import concourse.bacc as bacc
import concourse.bass as bass
from concourse import bass_utils, mybir
from concourse.tile import TileContext

F32 = mybir.dt.float32
BF16 = mybir.dt.bfloat16
AF = mybir.ActivationFunctionType
ALU = mybir.AluOpType
AX = mybir.AxisListType

B, P, N, DIM = 4, 1024, 32, 256
HEADS, DH, INNER = 8, 64, 512
SCALE = DH ** -0.5
NCORES = 8
GROUPS = B * P                   # 4096 independent attention groups
GPC = GROUPS // NCORES           # 512 groups per core
BLK = 128                        # tokens per block = 4 groups
GPB = BLK // N                   # 4 groups per block


def build_kernel_body(tc, x_d, wqkv_d, wout_d, bout_d, out_d, nblk):
    nc = tc.nc

    # ---------------- one-time weight prep ----------------
    with tc.tile_pool(name="wpool", bufs=1) as wp:
        # W_qkv [256, 1536] -> [128 part, dchunk 2, 1536] bf16
        wqkv_f = wp.tile([128, 2, 3 * INNER], F32, name="wqkv_f")
        nc.sync.dma_start(out=wqkv_f, in_=wqkv_d.rearrange("(c p) f -> p c f", c=2))
        wqkv_b = wp.tile([128, 2, 3 * INNER], BF16, name="wqkv_b")
        nc.vector.tensor_copy(wqkv_b, wqkv_f)

        # W_out [512, 256] -> [128 part, chunk 4, 256] bf16
        wout_f = wp.tile([128, 4, DIM], F32, name="wout_f")
        nc.sync.dma_start(out=wout_f, in_=wout_d.rearrange("(c p) f -> p c f", c=4))
        wout_b = wp.tile([128, 4, DIM], BF16, name="wout_b")
        nc.vector.tensor_copy(wout_b, wout_f)

        # bias replicated across partitions [128, 256] f32
        bias_t = wp.tile([128, DIM], F32, name="bias_t")
        nc.sync.dma_start(out=bias_t, in_=bout_d.unsqueeze(0).broadcast_to([128, DIM]))

        # identities for PE-mode transposes
        from concourse.masks import make_identity
        ident_f = wp.tile([128, 128], F32, name="ident_f")
        make_identity(nc, ident_f)
        ident_b = wp.tile([128, 128], BF16, name="ident_b")
        make_identity(nc, ident_b)

        _main_loop(tc, x_d, out_d, nblk, wqkv_b, wout_b, bias_t,
                   ident_f, ident_b)


def _main_loop(tc, x_d, out_d, nblk, wqkv_b, wout_b, bias_t,
               ident_f, ident_b):
    nc = tc.nc
    assert nblk % 2 == 0
    # x viewed as [pair, token-in-block 128, block-in-pair 2, 256]
    xv2 = x_d.rearrange("(n b p) d -> n p b d", b=2, p=BLK)
    ov2 = out_d.rearrange("(n b p) d -> n p b d", b=2, p=BLK)

    with (
        tc.tile_pool(name="io", bufs=4) as iop,
        tc.tile_pool(name="work", bufs=4) as wk,
        tc.tile_pool(name="ps_qkv", bufs=2, space="PSUM") as pqkv,
        tc.tile_pool(name="ps_attn", bufs=2, space="PSUM") as pat,
    ):
        state = {}

        def stage_a(i):
            # ---- load x (one SWDGE DMA per 2 blocks) ----
            if i % 2 == 0:
                state["x_f2"] = iop.tile([128, 2, DIM], F32, tag="x_f2",
                                         name="x_f2")
                nc.gpsimd.dma_start(out=state["x_f2"], in_=xv2[i // 2])
            x_f2 = state["x_f2"]

            qk_ps = pqkv.tile([128, 1024], F32, tag="qk_ps", name="qk_ps")
            attn_ps = pat.tile([128, 1024], F32, tag="attn_ps", name="attn_ps")

            # ---- transpose x via PE (fp32); evacuation does the bf16 cast
            for dc in range(2):
                nc.tensor.transpose(
                    qk_ps[:, 128 * dc:128 * dc + 128],
                    x_f2[:, i % 2, 128 * dc:128 * dc + 128], ident_f)
            xT = wk.tile([128, 2, 128], BF16, tag="xT", name="xT")
            nc.scalar.copy(xT.rearrange("p a b -> p (a b)"), qk_ps[:, 0:256])

            # ---- qkv projection ----
            # q,k feature-major into a 2-bank tile; v token-major goes into
            # bank 1 of attn_ps (its evac precedes any dots write there).
            for c in range(8):
                for dc in range(2):
                    nc.tensor.matmul(
                        qk_ps[:, 128 * c:128 * c + 128],
                        lhsT=wqkv_b[:, dc, 128 * c:128 * c + 128],
                        rhs=xT[:, dc],
                        start=(dc == 0), stop=(dc == 1))
            for dc in range(2):
                nc.tensor.matmul(
                    attn_ps[:, 512:1024],
                    lhsT=xT[:, dc],
                    rhs=wqkv_b[:, dc, 2 * INNER:3 * INNER],
                    start=(dc == 0), stop=(dc == 1))

            # split evacuation: ACT takes q then v, DVE takes k (parallel)
            qkv_sb = wk.tile([128, 1536], BF16, tag="qkv_sb", name="qkv_sb")
            nc.scalar.copy(qkv_sb[:, 0:512], qk_ps[:, 0:512])
            nc.vector.tensor_copy(qkv_sb[:, 512:1024], qk_ps[:, 512:1024])
            nc.scalar.copy(qkv_sb[:, 1024:1536], attn_ps[:, 512:1024])
            return attn_ps, qkv_sb

        def stage_b(i, attn_ps, qkv_sb):
            q_sb = qkv_sb[:, 0:512]
            k_sb = qkv_sb[:, 512:1024]
            v_sb = qkv_sb[:, 1024:1536]

            # ---- dots: per (group g, head h) 32x32, packed via tile_position ----
            # q_sb layout: [part = feat within chunk, free = (chunk c, token)]
            # head h = 2c+p -> partitions 64p..64p+64 of chunk c.
            # One [128,512] psum tile is reused dots -> oT -> out-proj; the
            # WAR chains between those uses coincide with real data deps.
            # Concurrent PE sub-array tiles must never drain into the same
            # PSUM bank at the same partitions (HW fault).  dots tiles for
            # the two row-parities therefore land in different banks:
            # head h=2c+pp writes attn_ps[32g:+32, 512*pp + 32*c :+32].
            for h in range(HEADS):
                c, pp = h // 2, h % 2
                for g in range(GPB):
                    col = 128 * c + 32 * g
                    dcol = 512 * pp + 32 * c
                    nc.tensor.matmul(
                        attn_ps[32 * g:32 * g + 32, dcol:dcol + 32],
                        lhsT=q_sb[64 * pp:64 * pp + 64, col:col + 32],
                        rhs=k_sb[64 * pp:64 * pp + 64, col:col + 32],
                        start=True, stop=True,
                        tile_position=(64 * pp, 32 * g))

            # ---- softmax over j (free dim), segmented per head ----
            # em free layout: head h=2c+pp at col 128*pp + 32*c.
            em = wk.tile([128, 256], F32, tag="em", name="em")
            dots_view = attn_ps.rearrange(
                "p (b x) -> p b x", b=2)[:, :, 0:128]
            nc.scalar.activation(
                em.rearrange("p (b x) -> p b x", b=2),
                dots_view, AF.Exp, bias=0.0, scale=SCALE)
            s_t = wk.tile([128, 8], F32, tag="s_t", name="s_t")
            nc.vector.reduce_sum(
                s_t, em.rearrange("p (h j) -> p h j", h=HEADS), axis=AX.X)
            r_t = wk.tile([128, 8], F32, tag="r_t", name="r_t")
            nc.vector.reciprocal(r_t, s_t)
            attn_b = wk.tile([128, 256], BF16, tag="attn_b", name="attn_b")
            nc.vector.tensor_mul(
                attn_b.rearrange("p (h j) -> p h j", h=HEADS),
                em.rearrange("p (h j) -> p h j", h=HEADS),
                r_t.unsqueeze(2).broadcast_to([128, 8, 32]))

            # ---- transpose attn blocks (32x32) : [(g,i),(h,j)] -> [(g,j),(h,i)] ----
            attnT = wk.tile([128, 256], BF16, tag="attnT", name="attnT")
            nc.vector.transpose(attnT, attn_b)

            # ---- attn @ v -> o (token-major), diagonal slots (32g,32g) ----
            # Concurrent tiles (different g) drain to different partitions;
            # sequential heads reuse the same slot (HW-serialized).  Output
            # o[(g,i), 64h+dh] goes to bank 0 of attn_ps (WAR after exp).
            o_ps = attn_ps[:, 0:512]
            for h in range(HEADS):
                c, pp = h // 2, h % 2
                acol = 128 * pp + 32 * c
                for g in range(GPB):
                    nc.tensor.matmul(
                        o_ps[32 * g:32 * g + 32, 64 * h:64 * h + 64],
                        lhsT=attnT[32 * g:32 * g + 32, acol:acol + 32],
                        rhs=v_sb[32 * g:32 * g + 32, 64 * h:64 * h + 64],
                        start=True, stop=True,
                        tile_position=(32 * g, 32 * g))

            o_sb = wk.tile([128, 512], BF16, tag="o_sb", name="o_sb")
            nc.scalar.copy(o_sb, o_ps)
            # transpose o to inner-major via PE (4x 128x128), reusing bank 0
            # (bf16 pairs packed into fp32 PSUM cells via bitcast views)
            for c in range(4):
                nc.tensor.transpose(
                    attn_ps[:, 64 * c:64 * c + 64].bitcast(BF16),
                    o_sb[:, 128 * c:128 * c + 128], ident_b)
            oT_sb = wk.tile([128, 4, 128], BF16, tag="oT_sb", name="oT_sb")
            nc.vector.tensor_copy(
                oT_sb.rearrange("p a b -> p (a b)"),
                attn_ps[:, 0:256].bitcast(BF16))

            # ---- out projection: accumulate over 4 inner chunks ----
            op_ps = attn_ps[:, 512:768]
            for c in range(4):
                nc.tensor.matmul(
                    op_ps,
                    lhsT=oT_sb[:, c],
                    rhs=wout_b[:, c],
                    start=(c == 0), stop=(c == 3))

            if i % 2 == 0:
                state["out_sb2"] = iop.tile([128, 2, DIM], F32, tag="out_sb2",
                                            name="out_sb2")
            nc.vector.scalar_tensor_tensor(
                out=state["out_sb2"][:, i % 2], in0=op_ps, scalar=1.0,
                in1=bias_t, op0=ALU.mult, op1=ALU.add)
            if i % 2 == 1:
                nc.gpsimd.dma_start(out=ov2[i // 2], in_=state["out_sb2"])

        # software-skewed emission: block i+1's projection work is emitted
        # before block i's attention so the in-order PE queue can fill the
        # softmax/evac wait time of block i with block i+1's matmuls.
        prev = None
        for i in range(nblk):
            cur = stage_a(i)
            if prev is not None:
                stage_b(i - 1, *prev)
            prev = cur
        stage_b(nblk - 1, *prev)


def build(nblk):
    nc = bacc.Bacc("TRN2", target_bir_lowering=False, debug=False,
                   enable_asserts=False)
    tok = nblk * BLK
    x_d = nc.dram_tensor("x", [tok, DIM], F32, kind="ExternalInput").ap()
    wqkv_d = nc.dram_tensor("w_qkv", [DIM, 3 * INNER], F32,
                            kind="ExternalInput").ap()
    wout_d = nc.dram_tensor("w_out", [INNER, DIM], F32,
                            kind="ExternalInput").ap()
    bout_d = nc.dram_tensor("b_out", [DIM], F32, kind="ExternalInput").ap()
    out_d = nc.dram_tensor("out", [tok, DIM], F32, kind="ExternalOutput").ap()
    with TileContext(nc) as tc:
        build_kernel_body(tc, x_d, wqkv_d, wout_d, bout_d, out_d, nblk)
    nc.compile()
    return nc


_NC_CACHE = {}


def _get_nc(nblk):
    if nblk not in _NC_CACHE:
        _NC_CACHE[nblk] = build(nblk)
    return _NC_CACHE[nblk]


def kernel(x, W_qkv, W_out, b_out, trace=False):
    assert x.shape == (B, P, N, DIM)
    nblk = GPC * N // BLK        # 128 blocks/core
    nc = _get_nc(nblk)
    xf = np.ascontiguousarray(x.reshape(GROUPS * N, DIM).astype(np.float32))
    shards = xf.reshape(NCORES, GPC * N, DIM)
    in_maps = [
        {"x": shards[i], "w_qkv": np.asarray(W_qkv, np.float32),
         "w_out": np.asarray(W_out, np.float32),
         "b_out": np.asarray(b_out, np.float32)}
        for i in range(NCORES)
    ]
    res = bass_utils.run_bass_kernel_spmd(
        nc, in_maps, core_ids=list(range(NCORES)), trace=trace)
    out = np.concatenate([res.results[i]["out"] for i in range(NCORES)], axis=0)
    out = out.reshape(B, P, N, DIM).astype(np.float32)
    if trace:
        return out, res
    return out



# revision 5
# speedup vs baseline: 1.1093x; 1.1093x over previous
"""Trainium2 Bass kernel for nn_Attention_37074157699274 (v2).

Multi-head self-attention over tiny 32-token groups:
  x [4, 1024, 32, 256] -> qkv -> per-(b,p)-group 8-head attention -> out proj.

Data-parallel over the 4096 (b,p) groups across 8 NeuronCores (512
groups/core).  On-core, groups are processed in superblocks of 256 tokens
(= 8 groups = 2 partition-span "halves" of 128 tokens), with a 2-stage
software pipeline so every cross-engine dependency has about a round of
slack:

  round r: | x-DMA(r+2) | cast+xbar-transpose(r+1) | qkv(r) |
           | dots(r-1) H0 | attn@v(r-2) H0 | dots(r-1) H1 |
           | exp/sum/recip(r-1) | q-evac(r) | attn@v(r-2) H1 |
           | outproj(r-2) + bias + out-DMA | normalize+transpose(r-1) |

Engine assignment (ns per 256-token superblock, vs ~5400 PE):
  PE   qkv 4096r + dots 2048r + av 2048r + outproj 2048r = 5495
  ACT  v evacs 1222 + oT-H0 evac 611 + exp 611 + q evac 1038 = 3482
  DVE  k evac 1192 + sum 594 + recip 77 + oT-H1 evac 658 + bias-stt 658
       + 32x32-transpose 594 = 3773
  POOL x bf16 cast 806 + normalize-mul 1206 = 2012
  SP   x-in / xT-xbar / out DMAs (HWDGE)

Key structural points:
  - xT comes from the DMA XBAR (14 ns/16x128 tile), not PE: x is cast to
    bf16 on GPSIMD then transposed straight into SBUF.
  - attn@v produces oT (inner-major) directly: out[dh, i] = lhsT(v[j, dh])
    .T @ rhs(attnT[j, i]); no separate o transpose.  Per group, pp0 heads
    then pp1 heads, so same-PE-slot reloads serialize and co-resident
    tiles never drain to the same (bank, partition) range.
  - PSUM (exactly 8 banks): qk [128,8,256] (4) | v/oT time-shared via
    rotating "vot" tag, 2 x [128,512] (2) | dp [128,2,512] (2) holding
    dots (cols 0:256, parity-split banks) and outproj (cols 256:512).
  - outproj writes/stt reads go through the *current* round's dp
    acquisition so tag-rotation WAR deps cover all cross-round hazards.
"""

import numpy as np

import concourse.bacc as bacc
import concourse.bass as bass
from concourse import bass_utils, mybir
from concourse.tile import TileContext

F32 = mybir.dt.float32
BF16 = mybir.dt.bfloat16
AF = mybir.ActivationFunctionType
ALU = mybir.AluOpType
AX = mybir.AxisListType

B, P, N, DIM = 4, 1024, 32, 256
HEADS, DH, INNER = 8, 64, 512
SCALE = DH ** -0.5
NCORES = 8
GROUPS = B * P                   # 4096 independent attention groups
GPC = GROUPS // NCORES           # 512 groups per core
SBLK = 256                       # tokens per superblock (2 halves of 128)
GPH = 4                          # groups per half
# per-group head visit order: pp0 heads then pp1 heads (drain safety)
HORDER = [0, 2, 4, 6, 1, 3, 5, 7]


def build_kernel_body(tc, x_d, wqkv_d, wout_d, bout_d, out_d, nsblk):
    nc = tc.nc

    # ---------------- one-time weight prep ----------------
    # Weight DMA emission is deferred into the main loop so the first x
    # DMAs win the DMA engines; W_qkv lands right after them, and
    # W_out/bias (not needed until several rounds in) come last.
    with tc.tile_pool(name="wpool", bufs=1) as wp:
        wqkv_f = wp.tile([128, 2, 3 * INNER], F32, name="wqkv_f")
        wqkv_b = wp.tile([128, 2, 3 * INNER], BF16, name="wqkv_b")
        wout_f = wp.tile([128, 4, DIM], F32, name="wout_f")
        wout_b = wp.tile([128, 4, DIM], BF16, name="wout_b")
        bias_t = wp.tile([128, DIM], F32, name="bias_t")

        def late_weights():
            # W_qkv [256, 1536] -> [128 part, dc 2, 1536] bf16
            nc.sync.dma_start(out=wqkv_f,
                              in_=wqkv_d.rearrange("(c p) f -> p c f", c=2))
            nc.vector.tensor_copy(wqkv_b, wqkv_f)
            # W_out [512, 256] -> [128 part, cc 4, 256] bf16
            nc.sync.dma_start(out=wout_f,
                              in_=wout_d.rearrange("(c p) f -> p c f", c=4))
            nc.vector.tensor_copy(wout_b, wout_f)
            # bias replicated across partitions [128, 256] f32
            nc.sync.dma_start(out=bias_t,
                              in_=bout_d.unsqueeze(0).broadcast_to([128, DIM]))

        _main_loop(tc, x_d, out_d, nsblk, wqkv_b, wout_b, bias_t,
                   late_weights)


def _main_loop(tc, x_d, out_d, nsblk, wqkv_b, wout_b, bias_t, late_weights):
    nc = tc.nc
    # x viewed as [sblk, 128 tokens-in-half, half 2, 256]
    xv = x_d.rearrange("(s h p) d -> s p h d", h=2, p=128)
    ov = out_d.rearrange("(s h p) d -> s p h d", h=2, p=128)

    with (
        tc.tile_pool(name="io", bufs=2) as iop,
        tc.tile_pool(name="wk", bufs=2) as wk,
        tc.tile_pool(name="ps", bufs=1, space="PSUM") as ps,
    ):
        S = [dict() for _ in range(nsblk)]  # per-superblock tile stash

        def load_x(s):
            S[s]["xf"] = iop.tile([128, 2, DIM], F32, tag="xf", bufs=5,
                                  name="xf")
            nc.sync.dma_start(out=S[s]["xf"], in_=xv[s])

        def prep_xT(s):
            xf = S[s].pop("xf")
            xb = iop.tile([128, 2, DIM], BF16, tag="xb", bufs=6, name="xb")
            nc.scalar.copy(xb, xf)
            # xT[d, half, dc, t] = xb[t, half, 128*dc + d]
            xT = wk.tile([128, 2, 2, 128], BF16, tag="xT", bufs=4, name="xT")
            nc.sync.dma_start_transpose(out=xT, in_=xb)
            S[s]["xT"] = xT

        def _qk_wave(s, lo, evac, out_name):
            # one 4-chunk wave of the q/k projection into the 2-bank "qk"
            # tile; the two waves time-share it (rotation WAR via the evac)
            xT = S[s]["xT"]
            w_ps = ps.tile([128, 4, DIM], F32, tag="qk", name="w_ps")
            for i in range(4):
                cc = lo + i
                for dc in range(2):
                    nc.tensor.matmul(
                        w_ps[:, i].rearrange("p (h t) -> p h t", h=2),
                        lhsT=wqkv_b[:, dc, 128 * cc:128 * cc + 128],
                        rhs=xT[:, :, dc, :],
                        start=(dc == 0), stop=(dc == 1))
            sb = wk.tile([128, 4, 2, 128], BF16, tag=out_name, name=out_name)
            evac(sb.rearrange("p c h t -> p (c h t)"),
                 w_ps.rearrange("p c t -> p (c t)"))
            S[s][out_name] = sb

        def q_wave(s):
            _qk_wave(s, 0, nc.vector.tensor_copy, "q_sb")

        def k_wave(s):
            _qk_wave(s, 4, nc.scalar.copy, "k_sb")
            S[s].pop("xT")

        def v_wave(s):
            xT = S[s]["xT"]
            v_ps = [ps.tile([128, INNER], F32, tag="vps", bufs=2, name="v_ps")
                    for _ in range(2)]
            # v token-major per half: out[tok, feat 512]
            for h5 in range(2):
                for dc in range(2):
                    nc.tensor.matmul(
                        v_ps[h5],
                        lhsT=xT[:, h5, dc, :],
                        rhs=wqkv_b[:, dc, 2 * INNER:3 * INNER],
                        start=(dc == 0), stop=(dc == 1))
            v_sb = wk.tile([128, 2, INNER], BF16, tag="v_sb", bufs=5,
                           name="v_sb")
            nc.scalar.copy(v_sb[:, 0], v_ps[0])
            nc.scalar.copy(v_sb[:, 1], v_ps[1])
            S[s]["v_sb"] = v_sb

        def dots_half(s, dp, h5):
            q_sb, k_sb = S[s]["q_sb"], S[s]["k_sb"]
            # dp dots layout: [32g+i part, pp bank, 128*half + 32*c + j]
            for h in range(HEADS):
                c, pp = h // 2, h % 2
                for g in range(GPH):
                    col = 128 * h5 + 32 * c
                    nc.tensor.matmul(
                        dp[32 * g:32 * g + 32, pp, col:col + 32],
                        lhsT=q_sb[64 * pp:64 * pp + 64, c, h5,
                                  32 * g:32 * g + 32],
                        rhs=k_sb[64 * pp:64 * pp + 64, c, h5,
                                 32 * g:32 * g + 32],
                        start=True, stop=True,
                        tile_position=(64 * pp, 32 * g))

        def soft1(s, dp):
            # exp (ACT) -> row-sums (DVE) -> reciprocal (DVE).
            # bf16 em/sums: the packed 2-byte operands give the DVE reduce
            # its 2x mode; precision cost is ~0.4% on attn, well in budget.
            em = wk.tile([128, 2, 2, 128], BF16, tag="em", name="em")
            nc.scalar.activation(
                em, dp[:, :, 0:256].rearrange("p b (h x) -> p b h x", h=2),
                AF.Exp, bias=0.0, scale=SCALE)
            s_t = wk.tile([128, 16], BF16, tag="s_t", name="s_t")
            with nc.allow_low_precision(reason="32-term bf16 row sums stay "
                                        "well within the 2e-2 budget"):
                nc.vector.reduce_sum(
                    s_t, em.rearrange("p a b (c j) -> p (a b c) j", j=32),
                    axis=AX.X)
            r_t = wk.tile([128, 16], F32, tag="r_t", name="r_t")
            nc.vector.reciprocal(r_t, s_t)
            S[s]["em"] = em
            S[s]["r_t"] = r_t

        def soft2(s):
            # normalize (GPSIMD) -> 32x32 block transpose (DVE)
            em, r_t = S[s].pop("em"), S[s].pop("r_t")
            attn_b = wk.tile([128, 16, 32], BF16, tag="attn_b", name="attn_b")
            nc.vector.tensor_mul(
                attn_b,
                em.rearrange("p a b (c j) -> p (a b c) j", j=32),
                r_t.unsqueeze(2).broadcast_to([128, 16, 32]))
            attnT = wk.tile([128, 512], BF16, tag="attnT", bufs=4,
                            name="attnT")
            nc.vector.transpose(attnT, attn_b.rearrange("p f j -> p (f j)"))
            S[s]["attnT"] = attnT

        def av_half(s, h5):
            # attn @ v, token-major: o[(g,i), (h,dh)] with diagonal 32x32
            # PE tiles (the only packed-tile shape validated on real TRN2
            # besides the 64x32 dots tiles).  o then goes through the DMA
            # XBAR to become oT[(pp,dh), (cc, tok)] for the out projection
            # -- straight into SBUF, no PE transpose, no extra evac.
            v_sb = S[s]["v_sb"]
            o_ps = ps.tile([128, INNER], F32, tag="otps", bufs=2,
                           name="o_ps")
            aT = S[s]["attnT"].rearrange("p (a b c i) -> p a b c i",
                                         a=2, b=2, c=4)
            for h in range(HEADS):
                c, pp = h // 2, h % 2
                for g in range(GPH):
                    nc.tensor.matmul(
                        o_ps[32 * g:32 * g + 32, 64 * h:64 * h + 64],
                        lhsT=aT[32 * g:32 * g + 32, pp, h5, c, :],
                        rhs=v_sb[32 * g:32 * g + 32, h5,
                                 64 * h:64 * h + 64],
                        start=True, stop=True,
                        tile_position=(32 * g, 32 * g))
            if h5 == 0:
                S[s]["o_sb"] = wk.tile([128, 2, INNER], BF16, tag="o_sb",
                                       bufs=3, name="o_sb")
                S[s]["oT_sb"] = wk.tile([128, 2, 4, 128], BF16, tag="oT_sb",
                                        bufs=3, name="oT_sb")
            nc.scalar.copy(S[s]["o_sb"][:, h5], o_ps)
            nc.sync.dma_start_transpose(out=S[s]["oT_sb"][:, h5],
                                        in_=S[s]["o_sb"][:, h5])

        def outproj(s, dp_cur):
            # accumulate over 4 inner chunks, both halves; writes/reads go
            # through the CURRENT round's dp acquisition (cols 256:512) so
            # tag-rotation WAR deps cover the cross-round hazards.
            oT_sb = S[s].pop("oT_sb")
            for h5 in range(2):
                for cc in range(4):
                    nc.tensor.matmul(
                        dp_cur[:, h5, 256:512],
                        lhsT=oT_sb[:, h5, cc, :],
                        rhs=wout_b[:, cc, :],
                        start=(cc == 0), stop=(cc == 3))
            S[s]["dp_op"] = dp_cur

        def finish_out(s):
            # bias-add + out-DMA at the TOP of the following round: the stt
            # is ready the moment DVE picks it up, so the op region frees
            # early and never back-pressures the next rounds.
            dp_op = S[s].pop("dp_op")
            out_sb = iop.tile([128, 2, DIM], F32, tag="out_sb", bufs=3,
                              name="out_sb")
            nc.vector.scalar_tensor_tensor(
                out=out_sb, in0=dp_op[:, :, 256:512], scalar=1.0,
                in1=bias_t.unsqueeze(1).broadcast_to([128, 2, DIM]),
                op0=ALU.mult, op1=ALU.add)
            nc.sync.dma_start(out=ov[s], in_=out_sb)
            S[s].clear()

        # -------- software-pipelined emission (2-stage skew) --------
        for s in range(min(4, nsblk)):
            load_x(s)
        late_weights()
        for s in range(min(3, nsblk)):
            prep_xT(s)
        for r in range(nsblk + 6):
            if 6 <= r < nsblk + 6:
                finish_out(r - 6)
            if r + 4 < nsblk:
                load_x(r + 4)
            if r < nsblk:
                q_wave(r)
                v_wave(r)
            dp = ps.tile([128, 2, INNER], F32, tag="dp", name="dp")
            if 1 <= r <= nsblk:
                dots_half(r - 1, dp, 0)
                dots_half(r - 1, dp, 1)
                soft1(r - 1, dp)
            if r < nsblk:
                k_wave(r)
            if 4 <= r < nsblk + 4:
                av_half(r - 4, 0)
                av_half(r - 4, 1)
            if 5 <= r < nsblk + 5:
                outproj(r - 5, dp)
            if 1 <= r <= nsblk:
                soft2(r - 1)
            if r + 3 < nsblk:
                prep_xT(r + 3)


def build(nsblk):
    nc = bacc.Bacc("TRN2", target_bir_lowering=False, debug=False,
                   enable_asserts=False)
    tok = nsblk * SBLK
    x_d = nc.dram_tensor("x", [tok, DIM], F32, kind="ExternalInput").ap()
    wqkv_d = nc.dram_tensor("w_qkv", [DIM, 3 * INNER], F32,
                            kind="ExternalInput").ap()
    wout_d = nc.dram_tensor("w_out", [INNER, DIM], F32,
                            kind="ExternalInput").ap()
    bout_d = nc.dram_tensor("b_out", [DIM], F32, kind="ExternalInput").ap()
    out_d = nc.dram_tensor("out", [tok, DIM], F32, kind="ExternalOutput").ap()
    with TileContext(nc) as tc:
        build_kernel_body(tc, x_d, wqkv_d, wout_d, bout_d, out_d, nsblk)
    nc.compile()
    return nc


_NC_CACHE = {}


def _get_nc(nsblk):
    if nsblk not in _NC_CACHE:
        _NC_CACHE[nsblk] = build(nsblk)
    return _NC_CACHE[nsblk]


def kernel(x, W_qkv, W_out, b_out, trace=False):
    assert x.shape == (B, P, N, DIM)
    nsblk = GPC * N // SBLK      # 64 superblocks/core
    nc = _get_nc(nsblk)
    xf = np.ascontiguousarray(x.reshape(GROUPS * N, DIM).astype(np.float32))
    shards = xf.reshape(NCORES, GPC * N, DIM)
    in_maps = [
        {"x": shards[i], "w_qkv": np.asarray(W_qkv, np.float32),
         "w_out": np.asarray(W_out, np.float32),
         "b_out": np.asarray(b_out, np.float32)}
        for i in range(NCORES)
    ]
    res = bass_utils.run_bass_kernel_spmd(
        nc, in_maps, core_ids=list(range(NCORES)), trace=trace)
    out = np.concatenate([res.results[i]["out"] for i in range(NCORES)], axis=0)
    out = out.reshape(B, P, N, DIM).astype(np.float32)
    if trace:
        return out, res
    return out


# revision 14
# speedup vs baseline: 1.1229x; 1.0122x over previous
"""Trainium2 Bass kernel for nn_Attention_37074157699274 (v2).

Multi-head self-attention over tiny 32-token groups:
  x [4, 1024, 32, 256] -> qkv -> per-(b,p)-group 8-head attention -> out proj.

Data-parallel over the 4096 (b,p) groups across 8 NeuronCores (512
groups/core).  On-core, groups are processed in superblocks of 256 tokens
(= 8 groups = 2 partition-span "halves" of 128 tokens), software-pipelined
about 6 rounds deep so every cross-engine dependency has at least a round
of slack:

  round r: | out(r-6) bias+DMA | x-DMA(r+4) | q-wave(r) | v-wave(r) |
           | dots(r-1) | exp/sum/recip(r-1) | k-wave(r) | attn@v(r-4) |
           | outproj(r-5) | normalize+transpose(r-1) | cast+xbar(r+3) |

Key structural points:
  - xT comes from the DMA XBAR (14 ns per 16x128 tile), not PE: x is cast
    to bf16 then transposed straight into SBUF.
  - attn@v is token-major with diagonal 32x32 PE tiles -- the only packed
    tile shape (besides the 64x32 dots tiles) that executes correctly on
    real TRN2; (32-row, 64-col) and full 4x4 32x32 grids fault the device.
  - o -> oT also goes through the DMA XBAR (one call per half), so the
    out projection needs no PE transpose and no extra PSUM evacuation.
  - softmax runs at [128, 512] superblock granularity: exp on ACT (fused
    *8^-0.5 scale, bf16 out), sum (2x packed mode) + reciprocal +
    normalize + 32x32 block transpose on DVE.
  - PSUM (exactly 8 banks): q/k waves time-share [128,4,256] (2 banks) |
    v [128,512] x2 (2) | o [128,512] x2 (2) | dp [128,2,512] (2) holding
    dots (cols 0:256, parity-split banks) and outproj (cols 256:512).
  - per 128 tokens PE does 3072 (qkv) + 1024 (dots) + 2048 (attn@v) +
    1024 (outproj) bf16 rows; every other engine is kept below that.
"""

import numpy as np

import concourse.bacc as bacc
import concourse.bass as bass
from concourse import bass_utils, mybir
from concourse.tile import TileContext

F32 = mybir.dt.float32
BF16 = mybir.dt.bfloat16
AF = mybir.ActivationFunctionType
ALU = mybir.AluOpType
AX = mybir.AxisListType

B, P, N, DIM = 4, 1024, 32, 256
HEADS, DH, INNER = 8, 64, 512
SCALE = DH ** -0.5
NCORES = 8
GROUPS = B * P                   # 4096 independent attention groups
GPC = GROUPS // NCORES           # 512 groups per core
SBLK = 256                       # tokens per superblock (2 halves of 128)
GPH = 4                          # groups per half
# per-group head visit order: pp0 heads then pp1 heads (drain safety)
HORDER = [0, 2, 4, 6, 1, 3, 5, 7]


def build_kernel_body(tc, x_d, wqkv_d, wout_d, bout_d, out_d, nsblk):
    nc = tc.nc

    # ---------------- one-time weight prep ----------------
    # Weight DMA emission is deferred into the main loop so the first x
    # DMAs win the DMA engines; W_qkv lands right after them, and
    # W_out/bias (not needed until several rounds in) come last.
    with tc.tile_pool(name="wpool", bufs=1) as wp:
        wqkv_f = wp.tile([128, 2, 3 * INNER], F32, name="wqkv_f")
        wqkv_b = wp.tile([128, 2, 3 * INNER], BF16, name="wqkv_b")
        wout_f = wp.tile([128, 4, DIM], F32, name="wout_f")
        wout_b = wp.tile([128, 4, DIM], BF16, name="wout_b")
        bias_t = wp.tile([128, DIM], F32, name="bias_t")

        def late_weights():
            # W_qkv [256, 1536] -> [128 part, dc 2, 1536] bf16
            nc.sync.dma_start(out=wqkv_f,
                              in_=wqkv_d.rearrange("(c p) f -> p c f", c=2))
            nc.vector.tensor_copy(wqkv_b, wqkv_f)
            # W_out [512, 256] -> [128 part, cc 4, 256] bf16
            nc.sync.dma_start(out=wout_f,
                              in_=wout_d.rearrange("(c p) f -> p c f", c=4))
            nc.vector.tensor_copy(wout_b, wout_f)
            # bias replicated across partitions [128, 256] f32
            nc.sync.dma_start(out=bias_t,
                              in_=bout_d.unsqueeze(0).broadcast_to([128, DIM]))

        _main_loop(tc, x_d, out_d, nsblk, wqkv_b, wout_b, bias_t,
                   late_weights)


def _main_loop(tc, x_d, out_d, nsblk, wqkv_b, wout_b, bias_t, late_weights):
    nc = tc.nc
    # x viewed as [sblk, 128 tokens-in-half, half 2, 256]
    xv = x_d.rearrange("(s h p) d -> s p h d", h=2, p=128)
    ov = out_d.rearrange("(s h p) d -> s p h d", h=2, p=128)

    with (
        tc.tile_pool(name="io", bufs=2) as iop,
        tc.tile_pool(name="wk", bufs=2) as wk,
        tc.tile_pool(name="ps", bufs=1, space="PSUM") as ps,
    ):
        S = [dict() for _ in range(nsblk)]  # per-superblock tile stash

        def load_x(s):
            S[s]["xf"] = iop.tile([128, 2, DIM], F32, tag="xf", bufs=5,
                                  name="xf")
            nc.sync.dma_start(out=S[s]["xf"], in_=xv[s])

        def prep_xT(s):
            xf = S[s].pop("xf")
            xb = iop.tile([128, 2, DIM], BF16, tag="xb", bufs=6, name="xb")
            nc.gpsimd.tensor_copy(xb, xf)
            # xT[d, half, dc, t] = xb[t, half, 128*dc + d]
            xT = wk.tile([128, 2, 2, 128], BF16, tag="xT", bufs=4, name="xT")
            nc.sync.dma_start_transpose(out=xT, in_=xb)
            S[s]["xT"] = xT

        def _qk_wave(s, lo, evac, out_name):
            # one 4-chunk wave of the q/k projection into the 2-bank "qk"
            # tile; the two waves time-share it (rotation WAR via the evac)
            xT = S[s]["xT"]
            w_ps = ps.tile([128, 4, DIM], F32, tag="qk", name="w_ps")
            for i in range(4):
                cc = lo + i
                for dc in range(2):
                    nc.tensor.matmul(
                        w_ps[:, i].rearrange("p (h t) -> p h t", h=2),
                        lhsT=wqkv_b[:, dc, 128 * cc:128 * cc + 128],
                        rhs=xT[:, :, dc, :],
                        start=(dc == 0), stop=(dc == 1))
            sb = wk.tile([128, 4, 2, 128], BF16, tag=out_name, name=out_name)
            evac(sb.rearrange("p c h t -> p (c h t)"),
                 w_ps.rearrange("p c t -> p (c t)"))
            S[s][out_name] = sb

        def q_wave(s):
            _qk_wave(s, 0, nc.vector.tensor_copy, "q_sb")

        def k_wave(s):
            _qk_wave(s, 4, nc.scalar.copy, "k_sb")
            S[s].pop("xT")

        def v_wave(s):
            xT = S[s]["xT"]
            v_ps = [ps.tile([128, INNER], F32, tag="vps", bufs=2, name="v_ps")
                    for _ in range(2)]
            # v token-major per half: out[tok, feat 512]
            for h5 in range(2):
                for dc in range(2):
                    nc.tensor.matmul(
                        v_ps[h5],
                        lhsT=xT[:, h5, dc, :],
                        rhs=wqkv_b[:, dc, 2 * INNER:3 * INNER],
                        start=(dc == 0), stop=(dc == 1))
            v_sb = wk.tile([128, 2, INNER], BF16, tag="v_sb", bufs=5,
                           name="v_sb")
            nc.scalar.copy(v_sb[:, 0], v_ps[0])
            nc.scalar.copy(v_sb[:, 1], v_ps[1])
            S[s]["v_sb"] = v_sb

        def dots_half(s, dp, h5):
            q_sb, k_sb = S[s]["q_sb"], S[s]["k_sb"]
            # dp dots layout: [32g+i part, pp bank, 128*half + 32*c + j]
            for h in range(HEADS):
                c, pp = h // 2, h % 2
                for g in range(GPH):
                    col = 128 * h5 + 32 * c
                    nc.tensor.matmul(
                        dp[32 * g:32 * g + 32, pp, col:col + 32],
                        lhsT=q_sb[64 * pp:64 * pp + 64, c, h5,
                                  32 * g:32 * g + 32],
                        rhs=k_sb[64 * pp:64 * pp + 64, c, h5,
                                 32 * g:32 * g + 32],
                        start=True, stop=True,
                        tile_position=(64 * pp, 32 * g))

        def soft1(s, dp):
            # exp (ACT) -> row-sums (DVE) -> reciprocal (DVE).
            # bf16 em/sums: the packed 2-byte operands give the DVE reduce
            # its 2x mode; precision cost is ~0.4% on attn, well in budget.
            em = wk.tile([128, 2, 2, 128], BF16, tag="em", name="em")
            nc.scalar.activation(
                em, dp[:, :, 0:256].rearrange("p b (h x) -> p b h x", h=2),
                AF.Exp, bias=0.0, scale=SCALE)
            s_t = wk.tile([128, 16], BF16, tag="s_t", name="s_t")
            with nc.allow_low_precision(reason="32-term bf16 row sums stay "
                                        "well within the 2e-2 budget"):
                nc.vector.reduce_sum(
                    s_t, em.rearrange("p a b (c j) -> p (a b c) j", j=32),
                    axis=AX.X)
            r_t = wk.tile([128, 16], F32, tag="r_t", name="r_t")
            nc.vector.reciprocal(r_t, s_t)
            S[s]["em"] = em
            S[s]["r_t"] = r_t

        def soft2(s):
            # normalize (GPSIMD) -> 32x32 block transpose (DVE)
            em, r_t = S[s].pop("em"), S[s].pop("r_t")
            attn_b = wk.tile([128, 16, 32], BF16, tag="attn_b", name="attn_b")
            nc.gpsimd.tensor_mul(
                attn_b,
                em.rearrange("p a b (c j) -> p (a b c) j", j=32),
                r_t.unsqueeze(2).broadcast_to([128, 16, 32]))
            attnT = wk.tile([128, 512], BF16, tag="attnT", bufs=4,
                            name="attnT")
            nc.vector.transpose(attnT, attn_b.rearrange("p f j -> p (f j)"))
            S[s]["attnT"] = attnT

        def av_half(s, h5):
            # attn @ v, token-major: o[(g,i), (h,dh)] with diagonal 32x32
            # PE tiles (the only packed-tile shape validated on real TRN2
            # besides the 64x32 dots tiles).  o then goes through the DMA
            # XBAR to become oT[(pp,dh), (cc, tok)] for the out projection
            # -- straight into SBUF, no PE transpose, no extra evac.
            v_sb = S[s]["v_sb"]
            o_ps = ps.tile([128, INNER], F32, tag="otps", bufs=2,
                           name="o_ps")
            aT = S[s]["attnT"].rearrange("p (a b c i) -> p a b c i",
                                         a=2, b=2, c=4)
            for h in range(HEADS):
                c, pp = h // 2, h % 2
                for g in range(GPH):
                    nc.tensor.matmul(
                        o_ps[32 * g:32 * g + 32, 64 * h:64 * h + 64],
                        lhsT=aT[32 * g:32 * g + 32, pp, h5, c, :],
                        rhs=v_sb[32 * g:32 * g + 32, h5,
                                 64 * h:64 * h + 64],
                        start=True, stop=True,
                        tile_position=(32 * g, 32 * g))
            if h5 == 0:
                S[s]["o_sb"] = wk.tile([128, 2, INNER], BF16, tag="o_sb",
                                       bufs=4, name="o_sb")
                S[s]["oT_sb"] = wk.tile([128, 2, 4, 128], BF16, tag="oT_sb",
                                        bufs=4, name="oT_sb")
            if h5 == 0:
                nc.scalar.copy(S[s]["o_sb"][:, h5], o_ps)
            else:
                nc.vector.tensor_copy(S[s]["o_sb"][:, h5], o_ps)
            nc.sync.dma_start_transpose(out=S[s]["oT_sb"][:, h5],
                                        in_=S[s]["o_sb"][:, h5])

        def outproj(s, dp_cur):
            # accumulate over 4 inner chunks, both halves; writes/reads go
            # through the CURRENT round's dp acquisition (cols 256:512) so
            # tag-rotation WAR deps cover the cross-round hazards.
            oT_sb = S[s].pop("oT_sb")
            for h5 in range(2):
                for cc in range(4):
                    nc.tensor.matmul(
                        dp_cur[:, h5, 256:512],
                        lhsT=oT_sb[:, h5, cc, :],
                        rhs=wout_b[:, cc, :],
                        start=(cc == 0), stop=(cc == 3))
            S[s]["dp_op"] = dp_cur

        def finish_out(s):
            # bias-add + out-DMA at the TOP of the following round: the stt
            # is ready the moment DVE picks it up, so the op region frees
            # early and never back-pressures the next rounds.
            dp_op = S[s].pop("dp_op")
            out_sb = iop.tile([128, 2, DIM], F32, tag="out_sb", bufs=3,
                              name="out_sb")
            nc.vector.scalar_tensor_tensor(
                out=out_sb, in0=dp_op[:, :, 256:512], scalar=1.0,
                in1=bias_t.unsqueeze(1).broadcast_to([128, 2, DIM]),
                op0=ALU.mult, op1=ALU.add)
            nc.sync.dma_start(out=ov[s], in_=out_sb)
            S[s].clear()

        # -------- software-pipelined emission (2-stage skew) --------
        for s in range(min(4, nsblk)):
            load_x(s)
        late_weights()
        for s in range(min(3, nsblk)):
            prep_xT(s)
        for r in range(nsblk + 7):
            if 7 <= r < nsblk + 7:
                finish_out(r - 7)
            if r + 4 < nsblk:
                load_x(r + 4)
            if r < nsblk:
                q_wave(r)
                v_wave(r)
            dp = ps.tile([128, 2, INNER], F32, tag="dp", name="dp")
            if 1 <= r <= nsblk:
                dots_half(r - 1, dp, 0)
                dots_half(r - 1, dp, 1)
                soft1(r - 1, dp)
            if r < nsblk:
                k_wave(r)
            if 4 <= r < nsblk + 4:
                av_half(r - 4, 0)
                av_half(r - 4, 1)
            if 6 <= r < nsblk + 6:
                outproj(r - 6, dp)
            if 1 <= r <= nsblk:
                soft2(r - 1)
            if r + 3 < nsblk:
                prep_xT(r + 3)


def build(nsblk):
    nc = bacc.Bacc("TRN2", target_bir_lowering=False, debug=False,
                   enable_asserts=False)
    tok = nsblk * SBLK
    x_d = nc.dram_tensor("x", [tok, DIM], F32, kind="ExternalInput").ap()
    wqkv_d = nc.dram_tensor("w_qkv", [DIM, 3 * INNER], F32,
                            kind="ExternalInput").ap()
    wout_d = nc.dram_tensor("w_out", [INNER, DIM], F32,
                            kind="ExternalInput").ap()
    bout_d = nc.dram_tensor("b_out", [DIM], F32, kind="ExternalInput").ap()
    out_d = nc.dram_tensor("out", [tok, DIM], F32, kind="ExternalOutput").ap()
    with TileContext(nc) as tc:
        build_kernel_body(tc, x_d, wqkv_d, wout_d, bout_d, out_d, nsblk)
    nc.compile()
    return nc


_NC_CACHE = {}


def _get_nc(nsblk):
    if nsblk not in _NC_CACHE:
        _NC_CACHE[nsblk] = build(nsblk)
    return _NC_CACHE[nsblk]


def kernel(x, W_qkv, W_out, b_out, trace=False):
    assert x.shape == (B, P, N, DIM)
    nsblk = GPC * N // SBLK      # 64 superblocks/core
    nc = _get_nc(nsblk)
    xf = np.ascontiguousarray(x.reshape(GROUPS * N, DIM).astype(np.float32))
    shards = xf.reshape(NCORES, GPC * N, DIM)
    in_maps = [
        {"x": shards[i], "w_qkv": np.asarray(W_qkv, np.float32),
         "w_out": np.asarray(W_out, np.float32),
         "b_out": np.asarray(b_out, np.float32)}
        for i in range(NCORES)
    ]
    res = bass_utils.run_bass_kernel_spmd(
        nc, in_maps, core_ids=list(range(NCORES)), trace=trace)
    out = np.concatenate([res.results[i]["out"] for i in range(NCORES)], axis=0)
    out = out.reshape(B, P, N, DIM).astype(np.float32)
    if trace:
        return out, res
    return out
